# revision 2
# baseline (speedup 1.0000x reference)
"""Trainium2 Bass kernel for nn_ConvWithFilter (per-location conv filters).

Computation: out[n, o, h, w] = relu( sum_k unfold(features)[n, l, k] *
filters[n, l, k, o] ),  l = h*W + w,  k in [0, C*3*3) ordered (c, kh, kw).

Strategy: filters dominate traffic (288 MiB f32 -> 144 MiB bf16). Shard
(n, l-quarter) across 8 cores -> 1024 locations/core, 18 MiB of bf16
filter stream per core. Host transposes each location's filter matrix to
o-major ([O, K] per location) so per-channel slices are contiguous.
Per 128-location block, DVE does the multiplies in 2x perf mode (big
tensor_tensor with the feature vector broadcast across channels); the
K=288 reductions are split across three engines:
  - A_ACT channels:  Act engine accumulate (activation Copy + accum_out)
  - P_POOL channels: Pool/GpSimd binary tree of adds
  - D_TTR channels:  DVE binary tree of adds (2x mode)
Trees finish with a small DVE tensor_reduce over width 9. fp32 accum,
ReLU on DVE, f32 output. Features are preloaded once per core; outputs
are batched into a single DMA.
"""

import numpy as np
import ml_dtypes

# Problem constants (hardcoded; kernel.py must be self-contained).
N, C, H, W = 2, 32, 64, 64
KSZ = 3
O = 32                 # out channels
K = C * KSZ * KSZ      # 288 contraction length
L = H * W              # 4096 locations
NCORES = 8
LSH = (N * L) // NCORES   # 1024 locations per core
P = 128                   # locations per block (SBUF partitions)
NBLK = LSH // P           # 8 blocks per core

BF16 = ml_dtypes.bfloat16

# Channel split across engines (sum must be O).
D_TTR = 8    # DVE tree-reduced channels
A_ACT = 10   # Act-accumulated channels
P_POOL = 14  # Pool tree-add channels

TRACE = False
TRACE_KW = {}

_CACHE = {}

TREE_WIDTHS = [144, 72, 36, 18, 9]


def _build_nc(repeat=1, d=D_TTR, a=A_ACT, p=P_POOL):
    from concourse import bacc, tile, mybir
    from contextlib import nullcontext

    assert d + a + p == O

    nc = bacc.Bacc("TRN2", debug=False)
    dt = mybir.dt

    filt = nc.dram_tensor("filt", [LSH, O * K], dt.bfloat16, kind="ExternalInput")
    feat = nc.dram_tensor("feat", [LSH, K], dt.bfloat16, kind="ExternalInput")
    out = nc.dram_tensor("out", [LSH, O], dt.float32, kind="ExternalOutput")

    filt_ap = filt.ap()
    feat_ap = feat.ap()
    out_ap = out.ap()

    def tree(eng, prod_view, nch, seg, scrpool, accum):
        """Binary tree of adds along K for nch channels; final 9-wide
        reduce + accumulate into `accum` on DVE."""
        cur = prod_view
        for li, w in enumerate(TREE_WIDTHS):
            t = scrpool.tile([P, nch * w], dt.bfloat16, tag=f"tr{seg}{li}")
            tv = t[:].rearrange("q (c k) -> q c k", k=w)
            eng.tensor_tensor(
                out=tv,
                in0=cur[:, :, :w],
                in1=cur[:, :, w : 2 * w],
                op=mybir.AluOpType.add,
            )
            cur = tv
        nc.vector.tensor_reduce(
            out=accum,
            in_=cur,
            axis=mybir.AxisListType.X,
            op=mybir.AluOpType.add,
        )

    with tile.TileContext(nc) as tc:
        rep_ctx = tc.For_i(0, repeat, 1) if repeat > 1 else nullcontext()
        with (
            tc.tile_pool(name="filtp", bufs=3) as filtp,
            tc.tile_pool(name="featp", bufs=2) as featp,
            tc.tile_pool(name="prodp", bufs=2) as prodp,
            tc.tile_pool(name="scrp", bufs=2) as scrp,
            tc.tile_pool(name="accp", bufs=3) as accp,
            tc.tile_pool(name="outp", bufs=2) as outp,
            rep_ctx,
        ):
            # Preload all features for this core: [P, NBLK, K]
            fe_all = featp.tile([P, NBLK * K], dt.bfloat16, tag="fe")
            nc.sync.dma_start(
                out=fe_all[:].rearrange("q (b k) -> q b k", k=K),
                in_=feat_ap.rearrange("(b q) k -> q b k", q=P),
            )
            out_all = outp.tile([P, NBLK * O], dt.float32, tag="oa")

            for b in range(NBLK):
                rows = slice(b * P, (b + 1) * P)
                ft = filtp.tile([P, O * K], dt.bfloat16, tag="ft")
                nc.sync.dma_start(out=ft[:], in_=filt_ap[rows, :])
                fe = fe_all[:, b * K : (b + 1) * K]

                # --- 2x multiplies: channels [d, O) first (feeds Act/Pool),
                #     then [0, d) for DVE's own tree ---
                prod = prodp.tile([P, O * K], dt.bfloat16, tag="prod")
                n_tt = a + p
                if n_tt:
                    nc.vector.tensor_tensor(
                        out=prod[:, d * K :].rearrange("q (c k) -> q c k", k=K),
                        in0=ft[:, d * K :].rearrange("q (c k) -> q c k", k=K),
                        in1=fe.broadcast_to((P, K, n_tt)).rearrange("q k c -> q c k"),
                        op=mybir.AluOpType.mult,
                    )
                if d:
                    nc.vector.tensor_tensor(
                        out=prod[:, : d * K].rearrange("q (c k) -> q c k", k=K),
                        in0=ft[:, : d * K].rearrange("q (c k) -> q c k", k=K),
                        in1=fe.broadcast_to((P, K, d)).rearrange("q k c -> q c k"),
                        op=mybir.AluOpType.mult,
                    )

                # --- Act accumulates channels [d, d+a) ---
                if a:
                    acc_a = accp.tile([P, a], dt.float32, tag="acc_a")
                    scr_a = scrp.tile([P, K], dt.bfloat16, tag="scr_a")
                    for i in range(a):
                        nc.scalar.activation(
                            out=scr_a[:],
                            in_=prod[:, (d + i) * K : (d + i + 1) * K],
                            func=mybir.ActivationFunctionType.Copy,
                            accum_out=acc_a[:, i : i + 1],
                        )

                # --- Pool tree for channels [d+a, O) ---
                if p:
                    acc_p = accp.tile([P, p], dt.float32, tag="acc_p")
                    tree(
                        nc.gpsimd,
                        prod[:, (d + a) * K :].rearrange("q (c k) -> q c k", k=K),
                        p, "p", scrp, acc_p[:],
                    )

                # --- DVE tree for channels [0, d) ---
                if d:
                    acc_d = accp.tile([P, d], dt.float32, tag="acc_d")
                    tree(
                        nc.vector,
                        prod[:, : d * K].rearrange("q (c k) -> q c k", k=K),
                        d, "d", scrp, acc_d[:],
                    )

                # --- ReLU into the batched output tile ---
                ob = out_all[:, b * O : (b + 1) * O]
                if d:
                    nc.vector.tensor_scalar_max(out=ob[:, 0:d], in0=acc_d[:], scalar1=0.0)
                if a:
                    nc.vector.tensor_scalar_max(
                        out=ob[:, d : d + a], in0=acc_a[:], scalar1=0.0
                    )
                if p:
                    nc.vector.tensor_scalar_max(
                        out=ob[:, d + a : O], in0=acc_p[:], scalar1=0.0
                    )

            nc.sync.dma_start(
                out=out_ap.rearrange("(b q) o -> q b o", q=P),
                in_=out_all[:].rearrange("q (b o) -> q b o", o=O),
            )
    nc.compile()
    return nc


def _build_null_nc():
    """Same ExternalInput/Output signature as _build_nc, minimal work.

    Used by test.py to subtract input-upload + dispatch overhead from the
    wall-clock SPMD time (no NTFF profiling hook under this axon build).
    """
    from concourse import bacc, tile, mybir

    nc = bacc.Bacc("TRN2", debug=False)
    dt = mybir.dt

    nc.dram_tensor("filt", [LSH, O * K], dt.bfloat16, kind="ExternalInput")
    feat = nc.dram_tensor("feat", [LSH, K], dt.bfloat16, kind="ExternalInput")
    out = nc.dram_tensor("out", [LSH, O], dt.float32, kind="ExternalOutput")

    with tile.TileContext(nc) as tc:
        with tc.tile_pool(name="np_", bufs=1) as pool:
            t = pool.tile([P, O], dt.float32, tag="t")
            nc.vector.memset(t[:], 0.0)
            nc.sync.dma_start(out=out.ap()[0:P, :], in_=t[:])
            # touch feat so the input isn't pruned
            tf = pool.tile([P, 8], dt.bfloat16, tag="tf")
            nc.sync.dma_start(out=tf[:], in_=feat.ap()[0:P, 0:8])
    nc.compile()
    return nc


def _unfold_np(x):
    """numpy mirror of the reference unfold: [N,C,H,W] -> [N, L, C*9]."""
    xp = np.pad(x, ((0, 0), (0, 0), (1, 1), (1, 1)))
    patches = [
        xp[:, :, i : i + H, j : j + W] for i in range(KSZ) for j in range(KSZ)
    ]
    unf = np.stack(patches, axis=2)          # [N, C, 9, H, W]
    unf = unf.reshape(N, K, L)               # k = c*9 + (kh*3+kw)
    return unf.transpose(0, 2, 1)            # [N, L, K]


def kernel(features: np.ndarray, filters: np.ndarray) -> np.ndarray:
    from concourse.bass_utils import run_bass_kernel_spmd

    features = np.asarray(features, dtype=np.float32)
    filters = np.asarray(filters, dtype=np.float32)

    feat_unf = _unfold_np(features)          # [N, L, K] f32
    filt_bf = filters.astype(BF16)           # [N, L, K, O]

    in_maps = []
    for core in range(NCORES):
        n, q = divmod(core, NCORES // N)
        sl = slice(q * LSH, (q + 1) * LSH)
        fe = np.ascontiguousarray(feat_unf[n, sl]).astype(BF16)
        # o-major: per location, filter matrix transposed to [O, K]
        ftT = np.ascontiguousarray(
            filt_bf[n, sl].transpose(0, 2, 1)
        ).reshape(LSH, O * K)
        in_maps.append({"filt": ftT, "feat": fe})

    if "nc" not in _CACHE:
        _CACHE["nc"] = _build_nc()
    _CACHE["in_maps"] = in_maps
    res = run_bass_kernel_spmd(
        _CACHE["nc"], in_maps, list(range(NCORES)), trace=TRACE, **TRACE_KW
    )
    _CACHE["last_result"] = res

    out = np.empty((N, O, H, W), np.float32)
    out_flat = out.reshape(N, O, L)
    for core in range(NCORES):
        n, q = divmod(core, NCORES // N)
        o = np.asarray(res.results[core]["out"], dtype=np.float32)  # [LSH, O]
        out_flat[n, :, q * LSH : (q + 1) * LSH] = o.T
    return out



# revision 16
# speedup vs baseline: 1.1183x; 1.1183x over previous
"""Trainium2 Bass kernel for nn_ConvWithFilter (per-location conv filters).

Computation: out[n, o, h, w] = relu( sum_k unfold(features)[n, l, k] *
filters[n, l, k, o] ),  l = h*W + w,  k in [0, C*3*3) ordered (c, kh, kw).

Strategy: filters dominate traffic (288 MiB f32 -> 144 MiB bf16). Shard
(n, l-quarter) across 8 cores -> 1024 locations/core, 18 MiB of bf16
filter stream per core -> DMA roofline ~57us/core at 332 GB/s.

Host transposes each location's filter matrix to o-major ([O, K] per
location), so each output channel's K=288 filter row is contiguous.

Primary kernel ("stt"): per 128-location block, one DVE
scalar_tensor_tensor per output channel computes
  dummy = filt_ch * fe ; acc[:, ch] = sum_K(dummy)
i.e. the multiply AND the full reduction in a single instruction.
InstTensorScalarPtr supports the DVE 2x/4x bf16 perf modes, so the
whole compute fits well under the per-block DMA slot on one engine with
zero cross-engine synchronization. fp32 accumulators, ReLU via
tensor_scalar_max, one batched output DMA.

Fallback kernel ("split", used if STT underperforms on real HW): DVE
multiplies into a prod buffer; reduction split across Act (Copy +
accum_out), Pool (binary add tree finished on Pool) and DVE (add tree +
tensor_reduce); relu on the owning engine.
"""

import numpy as np
import ml_dtypes

# Problem constants (hardcoded; kernel.py must be self-contained).
N, C, H, W = 2, 32, 64, 64
KSZ = 3
O = 32                 # out channels
K = C * KSZ * KSZ      # 288 contraction length
L = H * W              # 4096 locations
NCORES = 8
LSH = (N * L) // NCORES   # 1024 locations per core
P = 128                   # locations per block (SBUF partitions)
NBLK = LSH // P           # 8 blocks per core

BF16 = ml_dtypes.bfloat16

KERNEL_STYLE = "tsp"   # "tsp" | "split" | "stt"

# Channel split across engines (sum must be O). Column order [Act|Pool|DVE].
A_ACT = 11   # Act-accumulated channels
P_POOL = 10  # Pool-reduced channels (add tree)
D_TTR = 11   # DVE-reduced channels (TSP-accum, 4x)

# DMA chunk boundaries (channel counts) for the per-block filter stream.
CHUNKS = (A_ACT, P_POOL, D_TTR)

TRACE = False
TRACE_KW = {}

_CACHE = {}

TREE_WIDTHS = [144, 72, 36, 18, 9]


def _build_nc(repeat=1, style=None):
    from concourse import bacc, tile, mybir
    from contextlib import nullcontext

    style = style or KERNEL_STYLE

    nc = bacc.Bacc("TRN2", debug=False)
    dt = mybir.dt

    filt = nc.dram_tensor("filt", [LSH, O * K], dt.bfloat16, kind="ExternalInput")
    feat = nc.dram_tensor("feat", [LSH, K], dt.bfloat16, kind="ExternalInput")
    out = nc.dram_tensor("out", [LSH, O], dt.float32, kind="ExternalOutput")

    filt_ap = filt.ap()
    feat_ap = feat.ap()
    out_ap = out.ap()

    with tile.TileContext(nc) as tc:
        rep_ctx = tc.For_i(0, repeat, 1) if repeat > 1 else nullcontext()
        with (
            tc.tile_pool(name="filtp", bufs={"stt": 6, "tsp": 4}.get(style, 3)) as filtp,
            tc.tile_pool(name="featp", bufs=2) as featp,
            tc.tile_pool(name="scrp", bufs=2) as scrp,
            tc.tile_pool(name="accp", bufs=3) as accp,
            tc.tile_pool(name="outp", bufs=2) as outp,
            rep_ctx,
        ):
            # Features for this core, DMA'd per block just ahead of use so
            # the first multiply starts as early as possible: [P, NBLK, K]
            fe_all = featp.tile([P, NBLK * K], dt.bfloat16, tag="fe")
            out_all = outp.tile([P, NBLK * O], dt.float32, tag="oa")

            for b in range(NBLK):
                rows = slice(b * P, (b + 1) * P)
                nc.sync.dma_start(
                    out=fe_all[:, b * K : (b + 1) * K], in_=feat_ap[rows, :]
                )
                ft = filtp.tile([P, O * K], dt.bfloat16, tag="ft")
                # Chunked filter stream so compute starts early.
                c0 = 0
                for nch in CHUNKS:
                    c1 = c0 + nch * K
                    nc.sync.dma_start(
                        out=ft[:, c0:c1], in_=filt_ap[rows, c0:c1]
                    )
                    c0 = c1
                fe = fe_all[:, b * K : (b + 1) * K]
                ob = out_all[:, b * O : (b + 1) * O]

                if style == "stt":
                    _emit_block_stt(nc, tc, mybir, dt, scrp, accp, ft, fe, ob)
                elif style == "tsp":
                    _emit_block_tsp(nc, tc, mybir, dt, scrp, accp, ft, fe, ob)
                else:
                    _emit_block_split(nc, tc, mybir, dt, scrp, accp, ft, fe, ob)

            nc.sync.dma_start(
                out=out_ap.rearrange("(b q) o -> q b o", q=P),
                in_=out_all[:].rearrange("q (b o) -> q b o", o=O),
            )
    nc.compile()
    return nc


def _emit_block_stt(nc, tc, mybir, dt, scrp, accp, ft, fe, ob):
    """One fused multiply+reduce DVE instruction per output channel."""
    acc = accp.tile([P, O], dt.float32, tag="acc")
    scr = scrp.tile([P, K], dt.bfloat16, tag="scr")
    for o in range(O):
        nc.vector.scalar_tensor_tensor(
            out=scr[:],
            in0=ft[:, o * K : (o + 1) * K],
            scalar=1.0,
            in1=fe,
            op0=mybir.AluOpType.mult,
            op1=mybir.AluOpType.mult,
            accum_out=acc[:, o : o + 1],
        )
    nc.vector.tensor_scalar_max(out=ob[:], in0=acc[:], scalar1=0.0)


def _emit_block_tsp(nc, tc, mybir, dt, scrp, accp, ft, fe, ob):
    """DVE multiplies (2x bf16); per-channel reductions via
    tensor_scalar(accum_out) on DVE (4x perf mode), Act Copy-accum, and
    Pool TSP-accum. ReLU on the owning engine (a-group relu on Pool)."""
    a, p, d = A_ACT, P_POOL, D_TTR
    AK, PK = a * K, p * K
    AO = mybir.AluOpType

    def mult(prod_view, ft_view, nch):
        nc.vector.tensor_tensor(
            out=prod_view.rearrange("q (c k) -> q c k", k=K),
            in0=ft_view.rearrange("q (c k) -> q c k", k=K),
            in1=fe.broadcast_to((P, K, nch)).rearrange("q k c -> q c k"),
            op=AO.mult,
        )

    prod = scrp.tile([P, O * K], dt.bfloat16, tag="prod")
    mult(prod[:, :AK], ft[:, :AK], a)
    mult(prod[:, AK : AK + PK], ft[:, AK : AK + PK], p)
    mult(prod[:, AK + PK :], ft[:, AK + PK :], d)

    # Act accumulates channels [0, a)
    acc_a = accp.tile([P, a], dt.float32, tag="acc_a")
    scr_a = scrp.tile([P, K], dt.bfloat16, tag="scr_a")
    for i in range(a):
        nc.scalar.activation(
            out=scr_a[:],
            in_=prod[:, i * K : (i + 1) * K],
            func=mybir.ActivationFunctionType.Copy,
            accum_out=acc_a[:, i : i + 1],
        )

    # Pool binary add tree for channels [a, a+p), finished on Pool
    # (gpsimd codegen rejects the TSP-reduce form, so tree it is).
    def tree(eng, prod_view, nch, seg):
        cur = prod_view
        for li, w in enumerate(TREE_WIDTHS):
            t = scrp.tile([P, nch * w], dt.bfloat16, tag=f"tr{seg}{li}")
            tv = t[:].rearrange("q (c k) -> q c k", k=w)
            eng.tensor_tensor(
                out=tv, in0=cur[:, :, :w], in1=cur[:, :, w : 2 * w],
                op=AO.add,
            )
            cur = tv
        return cur

    q9 = tree(nc.gpsimd, prod[:, AK : AK + PK].rearrange("q (c k) -> q c k", k=K),
              p, "p")
    q4 = scrp.tile([P, p * 4], dt.bfloat16, tag="q4")
    q4v = q4[:].rearrange("q (c k) -> q c k", k=4)
    nc.gpsimd.tensor_tensor(out=q4v, in0=q9[:, :, 0:4], in1=q9[:, :, 4:8],
                            op=AO.add)
    q2 = scrp.tile([P, p * 2], dt.bfloat16, tag="q2")
    q2v = q2[:].rearrange("q (c k) -> q c k", k=2)
    nc.gpsimd.tensor_tensor(out=q2v, in0=q4v[:, :, 0:2], in1=q4v[:, :, 2:4],
                            op=AO.add)
    q1 = scrp.tile([P, p], dt.bfloat16, tag="q1")
    q1v = q1[:].rearrange("q (c k) -> q c k", k=1)
    nc.gpsimd.tensor_tensor(out=q1v, in0=q2v[:, :, 0:1], in1=q2v[:, :, 1:2],
                            op=AO.add)
    acc_p = scrp.tile([P, p], dt.bfloat16, tag="qa")
    qav = acc_p[:].rearrange("q (c k) -> q c k", k=1)
    nc.gpsimd.tensor_tensor(out=qav, in0=q1v, in1=q9[:, :, 8:9],
                            op=AO.add)

    # DVE TSP-accum for channels [a+p, O)
    acc_d = accp.tile([P, d], dt.float32, tag="acc_d")
    scr_d = scrp.tile([P, K], dt.bfloat16, tag="scr_d")
    for j in range(d):
        nc.vector.tensor_scalar(
            out=scr_d[:],
            in0=prod[:, (a + p + j) * K : (a + p + j + 1) * K],
            scalar1=1.0,
            scalar2=0.0,
            op0=AO.mult,
            op1=AO.add,
            accum_out=acc_d[:, j : j + 1],
        )

    # ReLU on the owning engine (never DVE<-Act/Pool)
    nc.vector.tensor_scalar_max(out=ob[:, a + p : O], in0=acc_d[:], scalar1=0.0)
    nc.gpsimd.tensor_scalar_max(out=ob[:, a : a + p], in0=acc_p[:], scalar1=0.0)
    nc.gpsimd.tensor_scalar_max(out=ob[:, 0:a], in0=acc_a[:], scalar1=0.0)


def _emit_block_split(nc, tc, mybir, dt, scrp, accp, ft, fe, ob):
    """3-engine reduction split fallback (DVE mults; Act/Pool/DVE reduce)."""
    a, p, d = A_ACT, P_POOL, D_TTR
    AK, PK = a * K, p * K

    def mult(prod_view, ft_view, nch):
        nc.vector.tensor_tensor(
            out=prod_view.rearrange("q (c k) -> q c k", k=K),
            in0=ft_view.rearrange("q (c k) -> q c k", k=K),
            in1=fe.broadcast_to((P, K, nch)).rearrange("q k c -> q c k"),
            op=mybir.AluOpType.mult,
        )

    def tree(eng, prod_view, nch, seg):
        cur = prod_view
        for li, w in enumerate(TREE_WIDTHS):
            t = scrp.tile([P, nch * w], dt.bfloat16, tag=f"tr{seg}{li}")
            tv = t[:].rearrange("q (c k) -> q c k", k=w)
            eng.tensor_tensor(
                out=tv, in0=cur[:, :, :w], in1=cur[:, :, w : 2 * w],
                op=mybir.AluOpType.add,
            )
            cur = tv
        return cur

    prod = scrp.tile([P, O * K], dt.bfloat16, tag="prod")
    mult(prod[:, :AK], ft[:, :AK], a)
    mult(prod[:, AK : AK + PK], ft[:, AK : AK + PK], p)
    mult(prod[:, AK + PK :], ft[:, AK + PK :], d)

    # Act accumulates channels [0, a)
    acc_a = accp.tile([P, a], dt.float32, tag="acc_a")
    scr_a = scrp.tile([P, K], dt.bfloat16, tag="scr_a")
    for i in range(a):
        nc.scalar.activation(
            out=scr_a[:],
            in_=prod[:, i * K : (i + 1) * K],
            func=mybir.ActivationFunctionType.Copy,
            accum_out=acc_a[:, i : i + 1],
        )

    # Pool tree for channels [a, a+p), finished on Pool
    q9 = tree(nc.gpsimd, prod[:, AK : AK + PK].rearrange("q (c k) -> q c k", k=K),
              p, "p")
    q4 = scrp.tile([P, p * 4], dt.bfloat16, tag="q4")
    q4v = q4[:].rearrange("q (c k) -> q c k", k=4)
    nc.gpsimd.tensor_tensor(out=q4v, in0=q9[:, :, 0:4], in1=q9[:, :, 4:8],
                            op=mybir.AluOpType.add)
    q2 = scrp.tile([P, p * 2], dt.bfloat16, tag="q2")
    q2v = q2[:].rearrange("q (c k) -> q c k", k=2)
    nc.gpsimd.tensor_tensor(out=q2v, in0=q4v[:, :, 0:2], in1=q4v[:, :, 2:4],
                            op=mybir.AluOpType.add)
    q1 = scrp.tile([P, p], dt.bfloat16, tag="q1")
    q1v = q1[:].rearrange("q (c k) -> q c k", k=1)
    nc.gpsimd.tensor_tensor(out=q1v, in0=q2v[:, :, 0:1], in1=q2v[:, :, 1:2],
                            op=mybir.AluOpType.add)
    qa = scrp.tile([P, p], dt.bfloat16, tag="qa")
    qav = qa[:].rearrange("q (c k) -> q c k", k=1)
    nc.gpsimd.tensor_tensor(out=qav, in0=q1v, in1=q9[:, :, 8:9],
                            op=mybir.AluOpType.add)

    # DVE tree for channels [a+p, O)
    acc_d = accp.tile([P, d], dt.float32, tag="acc_d")
    t9 = tree(nc.vector, prod[:, AK + PK :].rearrange("q (c k) -> q c k", k=K),
              d, "d")
    nc.vector.tensor_reduce(out=acc_d[:], in_=t9, axis=mybir.AxisListType.X,
                            op=mybir.AluOpType.add)

    # ReLU on the owning engine (never DVE<-Act/Pool)
    nc.vector.tensor_scalar_max(out=ob[:, a + p : O], in0=acc_d[:], scalar1=0.0)
    nc.gpsimd.tensor_scalar_max(out=ob[:, a : a + p], in0=qa[:], scalar1=0.0)
    nc.gpsimd.tensor_scalar_max(out=ob[:, 0:a], in0=acc_a[:], scalar1=0.0)


def _build_null_nc():
    """Same ExternalInput/Output signature as _build_nc, minimal work.

    Used by test.py to subtract input-upload + dispatch overhead from the
    wall-clock SPMD time (no NTFF profiling hook under this axon build).
    """
    from concourse import bacc, tile, mybir

    nc = bacc.Bacc("TRN2", debug=False)
    dt = mybir.dt

    nc.dram_tensor("filt", [LSH, O * K], dt.bfloat16, kind="ExternalInput")
    feat = nc.dram_tensor("feat", [LSH, K], dt.bfloat16, kind="ExternalInput")
    out = nc.dram_tensor("out", [LSH, O], dt.float32, kind="ExternalOutput")

    with tile.TileContext(nc) as tc:
        with tc.tile_pool(name="np_", bufs=1) as pool:
            t = pool.tile([P, O], dt.float32, tag="t")
            nc.vector.memset(t[:], 0.0)
            nc.sync.dma_start(out=out.ap()[0:P, :], in_=t[:])
            # touch feat so the input isn't pruned
            tf = pool.tile([P, 8], dt.bfloat16, tag="tf")
            nc.sync.dma_start(out=tf[:], in_=feat.ap()[0:P, 0:8])
    nc.compile()
    return nc


def _unfold_np(x):
    """numpy mirror of the reference unfold: [N,C,H,W] -> [N, L, C*9]."""
    xp = np.pad(x, ((0, 0), (0, 0), (1, 1), (1, 1)))
    patches = [
        xp[:, :, i : i + H, j : j + W] for i in range(KSZ) for j in range(KSZ)
    ]
    unf = np.stack(patches, axis=2)          # [N, C, 9, H, W]
    unf = unf.reshape(N, K, L)               # k = c*9 + (kh*3+kw)
    return unf.transpose(0, 2, 1)            # [N, L, K]


def kernel(features: np.ndarray, filters: np.ndarray) -> np.ndarray:
    from concourse.bass_utils import run_bass_kernel_spmd

    features = np.asarray(features, dtype=np.float32)
    filters = np.asarray(filters, dtype=np.float32)

    feat_unf = _unfold_np(features)          # [N, L, K] f32
    filt_bf = filters.astype(BF16)           # [N, L, K, O]

    in_maps = []
    for core in range(NCORES):
        n, q = divmod(core, NCORES // N)
        sl = slice(q * LSH, (q + 1) * LSH)
        fe = np.ascontiguousarray(feat_unf[n, sl]).astype(BF16)
        # o-major: per location, filter matrix transposed to [O, K]
        ftT = np.ascontiguousarray(
            filt_bf[n, sl].transpose(0, 2, 1)
        ).reshape(LSH, O * K)
        in_maps.append({"filt": ftT, "feat": fe})

    if "nc" not in _CACHE:
        _CACHE["nc"] = _build_nc()
    _CACHE["in_maps"] = in_maps
    res = run_bass_kernel_spmd(
        _CACHE["nc"], in_maps, list(range(NCORES)), trace=TRACE, **TRACE_KW
    )
    _CACHE["last_result"] = res

    out = np.empty((N, O, H, W), np.float32)
    out_flat = out.reshape(N, O, L)
    for core in range(NCORES):
        n, q = divmod(core, NCORES // N)
        o = np.asarray(res.results[core]["out"], dtype=np.float32)  # [LSH, O]
        out_flat[n, :, q * LSH : (q + 1) * LSH] = o.T
    return out


# revision 35
# speedup vs baseline: 1.5176x; 1.3570x over previous
"""Trainium2 Bass kernel for nn_ConvWithFilter (per-location conv filters).

Computation: out[n, o, h, w] = relu( sum_k unfold(features)[n, l, k] *
filters[n, l, k, o] ),  l = h*W + w,  k in [0, C*3*3) ordered (c, kh, kw).

Strategy: filters dominate traffic (288 MiB f32 -> 144 MiB bf16). Shard
(n, l-quarter) across 8 cores -> 1024 locations/core, 18 MiB of bf16
filter stream per core -> DMA roofline ~57us/core at 332 GB/s.

Host transposes each location's filter matrix to o-major ([O, K] per
location), so each output channel's K=288 filter row is contiguous.

Primary kernel ("stt"): per 128-location block, one DVE
scalar_tensor_tensor per output channel computes
  dummy = filt_ch * fe ; acc[:, ch] = sum_K(dummy)
i.e. the multiply AND the full reduction in a single instruction.
InstTensorScalarPtr supports the DVE 2x/4x bf16 perf modes, so the
whole compute fits well under the per-block DMA slot on one engine with
zero cross-engine synchronization. fp32 accumulators, ReLU via
tensor_scalar_max, one batched output DMA.

Fallback kernel ("split", used if STT underperforms on real HW): DVE
multiplies into a prod buffer; reduction split across Act (Copy +
accum_out), Pool (binary add tree finished on Pool) and DVE (add tree +
tensor_reduce); relu on the owning engine.
"""

import numpy as np
import ml_dtypes

# Problem constants (hardcoded; kernel.py must be self-contained).
N, C, H, W = 2, 32, 64, 64
KSZ = 3
O = 32                 # out channels
K = C * KSZ * KSZ      # 288 contraction length
L = H * W              # 4096 locations
NCORES = 8
LSH = (N * L) // NCORES   # 1024 locations per core
P = 128                   # locations per block (SBUF partitions)
NBLK = LSH // P           # 8 blocks per core

BF16 = ml_dtypes.bfloat16

KERNEL_STYLE = "tsp"   # "tsp" | "split" | "stt"

# Channel split across engines (sum must be O). Column order [Act|Pool|DVE].
A_ACT = 11   # Act-accumulated channels
P_POOL = 0   # Pool-started tree channels (0 = Pool unused)
D_TTR = 21   # DVE full-tree channels

# DMA chunk boundaries (channel counts) for the per-block filter stream.
CHUNKS = (A_ACT, P_POOL, D_TTR)

TRACE = False
TRACE_KW = {}

_CACHE = {}

TREE_WIDTHS = [144, 72, 36, 18, 9]
SCRP_BUFS = 2
POOL_LEVELS = 2   # tree levels Pool runs for its channels before DVE takes over


def _build_nc(repeat=1, style=None):
    from concourse import bacc, tile, mybir
    from contextlib import nullcontext

    style = style or KERNEL_STYLE

    nc = bacc.Bacc("TRN2", debug=False)
    dt = mybir.dt

    filt = nc.dram_tensor("filt", [LSH, O * K], dt.bfloat16, kind="ExternalInput")
    feat = nc.dram_tensor("feat", [LSH, K], dt.bfloat16, kind="ExternalInput")
    out = nc.dram_tensor("out", [LSH, O], dt.float32, kind="ExternalOutput")

    filt_ap = filt.ap()
    feat_ap = feat.ap()
    out_ap = out.ap()

    with tile.TileContext(nc) as tc:
        rep_ctx = tc.For_i(0, repeat, 1) if repeat > 1 else nullcontext()
        with (
            tc.tile_pool(name="filtp", bufs={"stt": 6, "tsp": 4}.get(style, 3)) as filtp,
            tc.tile_pool(name="featp", bufs=2) as featp,
            tc.tile_pool(name="prodp", bufs=3) as prodp,
            tc.tile_pool(name="scrp", bufs=SCRP_BUFS) as scrp,
            tc.tile_pool(name="accp", bufs=3) as accp,
            tc.tile_pool(name="outp", bufs=2) as outp,
            rep_ctx,
        ):
            # Features for this core, DMA'd per block just ahead of use so
            # the first multiply starts as early as possible: [P, NBLK, K]
            fe_all = featp.tile([P, NBLK * K], dt.bfloat16, tag="fe")
            out_all = outp.tile([P, NBLK * O], dt.float32, tag="oa")
            if style.startswith("abl:"):
                # ablation kernels may leave out_all (partially) unwritten
                nc.vector.memset(out_all[:], 0.0)

            for b in range(NBLK):
                rows = slice(b * P, (b + 1) * P)
                nc.sync.dma_start(
                    out=fe_all[:, b * K : (b + 1) * K], in_=feat_ap[rows, :]
                )
                ft = filtp.tile([P, O * K], dt.bfloat16, tag="ft")
                # Chunked filter stream so compute starts early.
                c0 = 0
                for nch in CHUNKS:
                    if nch == 0:
                        continue
                    c1 = c0 + nch * K
                    nc.sync.dma_start(
                        out=ft[:, c0:c1], in_=filt_ap[rows, c0:c1]
                    )
                    c0 = c1
                fe = fe_all[:, b * K : (b + 1) * K]
                ob = out_all[:, b * O : (b + 1) * O]

                if style == "stt":
                    _emit_block_stt(nc, tc, mybir, dt, scrp, accp, ft, fe, ob)
                elif style == "tsp":
                    carry = _emit_block_pipe(nc, mybir, dt, prodp, scrp, accp,
                                             ft, fe, ob, carry if b else None)
                elif style.startswith("abl:"):
                    # ablation: "abl:" (DMA only), "abl:mult", "abl:mult,dve", ...
                    parts = tuple(x for x in style[4:].split(",") if x)
                    _emit_block_tsp(nc, tc, mybir, dt, prodp, scrp, accp, ft, fe, ob,
                                    parts=parts)
                else:
                    _emit_block_split(nc, tc, mybir, dt, scrp, accp, ft, fe, ob)

            if style == "tsp":
                _emit_pipe_drain(nc, mybir, dt, scrp, carry)

            nc.sync.dma_start(
                out=out_ap.rearrange("(b q) o -> q b o", q=P),
                in_=out_all[:].rearrange("q (b o) -> q b o", o=O),
            )
    nc.compile()
    return nc


def _emit_block_stt(nc, tc, mybir, dt, scrp, accp, ft, fe, ob):
    """One fused multiply+reduce DVE instruction per output channel."""
    acc = accp.tile([P, O], dt.float32, tag="acc")
    scr = scrp.tile([P, K], dt.bfloat16, tag="scr")
    for o in range(O):
        nc.vector.scalar_tensor_tensor(
            out=scr[:],
            in0=ft[:, o * K : (o + 1) * K],
            scalar=1.0,
            in1=fe,
            op0=mybir.AluOpType.mult,
            op1=mybir.AluOpType.mult,
            accum_out=acc[:, o : o + 1],
        )
    nc.vector.tensor_scalar_max(out=ob[:], in0=acc[:], scalar1=0.0)


def _emit_block_pipe(nc, mybir, dt, prodp, scrp, accp, ft, fe, ob, carry):
    """Software-pipelined block: DVE consumes Pool's partial tree and Act's
    accumulators from the PREVIOUS block, so no engine ever waits on a
    same-block cross-engine result (the in-order sequencers would stall).

    Per block b:
      DVE : mults(b) | d-tree(b)+reduce+relu | p-cont(b-1)+reduce+relu |
            relu_a(b-1)
      Act : a accumulates(b)
      Pool: first POOL_LEVELS tree levels for p channels(b)
    Returns carry = (qp, acc_a, ob) for block b; pass the previous carry in.
    """
    a, p, d = A_ACT, P_POOL, D_TTR
    AK, PK = a * K, p * K
    AO = mybir.AluOpType

    prod = prodp.tile([P, O * K], dt.bfloat16, tag="prod")
    for lo, nch in ((0, a), (AK, p), (AK + PK, d)):
        if nch:
            nc.vector.tensor_tensor(
                out=prod[:, lo : lo + nch * K].rearrange("q (c k) -> q c k", k=K),
                in0=ft[:, lo : lo + nch * K].rearrange("q (c k) -> q c k", k=K),
                in1=fe.broadcast_to((P, K, nch)).rearrange("q k c -> q c k"),
                op=AO.mult,
            )

    def tree(eng, cur, nch, seg, levels):
        for li, w in enumerate(levels):
            t = scrp.tile([P, nch * w], dt.bfloat16, tag=f"tr{seg}{li}")
            tv = t[:].rearrange("q (c k) -> q c k", k=w)
            eng.tensor_tensor(out=tv, in0=cur[:, :, :w],
                              in1=cur[:, :, w : 2 * w], op=AO.add)
            cur = tv
        return cur

    # Shared accumulator for the whole block: Act writes cols [0, a),
    # DVE cols [a+p, O); the previous block's Pool channels land in
    # [a, a+p) during the next block. One TSP relu covers all 32.
    acc = accp.tile([P, O], dt.float32, tag="acc")

    # Act: accumulate a channels of this block
    scr_a0 = scrp.tile([P, K], dt.bfloat16, tag="scr_a0")
    scr_a1 = scrp.tile([P, K], dt.bfloat16, tag="scr_a1")
    scr_as = (scr_a0, scr_a1)
    for i in range(a):
        nc.scalar.activation(
            out=scr_as[i % 2][:],
            in_=prod[:, i * K : (i + 1) * K],
            func=mybir.ActivationFunctionType.Copy,
            accum_out=acc[:, i : i + 1],
        )

    # Pool: first POOL_LEVELS levels for p channels of this block
    qp = None
    if p:
        qp = tree(nc.gpsimd,
                  prod[:, AK : AK + PK].rearrange("q (c k) -> q c k", k=K),
                  p, "p", TREE_WIDTHS[:POOL_LEVELS])

    # DVE: own d channels of this block (tree + reduce)
    if d:
        t9d = tree(nc.vector,
                   prod[:, AK + PK :].rearrange("q (c k) -> q c k", k=K),
                   d, "d", TREE_WIDTHS)
        nc.vector.tensor_reduce(out=acc[:, a + p : O], in_=t9d,
                                axis=mybir.AxisListType.X, op=AO.add)

    # DVE: finish the PREVIOUS block (Pool tail + one relu for all 32)
    if carry is not None:
        _emit_pipe_drain(nc, mybir, dt, scrp, carry)

    return (qp, acc, ob)


def _emit_pipe_drain(nc, mybir, dt, scrp, carry):
    """DVE-side completion of one block: Pool-channel tail + single relu."""
    a, p, d = A_ACT, P_POOL, D_TTR
    AO = mybir.AluOpType
    qp, acc, ob = carry

    def tree(cur, nch, seg, levels):
        for li, w in enumerate(levels):
            t = scrp.tile([P, nch * w], dt.bfloat16, tag=f"tr{seg}{li}")
            tv = t[:].rearrange("q (c k) -> q c k", k=w)
            nc.vector.tensor_tensor(out=tv, in0=cur[:, :, :w],
                                    in1=cur[:, :, w : 2 * w], op=AO.add)
            cur = tv
        return cur

    if p:
        t9p = tree(qp, p, "pc", TREE_WIDTHS[POOL_LEVELS:])
        nc.vector.tensor_reduce(out=acc[:, a : a + p], in_=t9p,
                                axis=mybir.AxisListType.X, op=AO.add)
    nc.vector.tensor_scalar_max(out=ob[:], in0=acc[:], scalar1=0.0)


def _emit_block_tsp(nc, tc, mybir, dt, prodp, scrp, accp, ft, fe, ob,
                    parts=("mult", "act", "pool", "dve")):
    """DVE multiplies (2x bf16); per-channel reductions via
    tensor_scalar(accum_out) on DVE (4x perf mode), Act Copy-accum, and
    Pool add tree. ReLU on the owning engine (a-group relu on Pool).
    `parts` restricts the emitted compute for HW ablation timing."""
    a, p, d = A_ACT, P_POOL, D_TTR
    AK, PK = a * K, p * K
    AO = mybir.AluOpType

    def mult(prod_view, ft_view, nch):
        nc.vector.tensor_tensor(
            out=prod_view.rearrange("q (c k) -> q c k", k=K),
            in0=ft_view.rearrange("q (c k) -> q c k", k=K),
            in1=fe.broadcast_to((P, K, nch)).rearrange("q k c -> q c k"),
            op=AO.mult,
        )

    if "mult" not in parts:
        return
    prod = prodp.tile([P, O * K], dt.bfloat16, tag="prod")
    mult(prod[:, :AK], ft[:, :AK], a)
    if p:
        mult(prod[:, AK : AK + PK], ft[:, AK : AK + PK], p)
    mult(prod[:, AK + PK :], ft[:, AK + PK :], d)

    if "act" in parts:
        # Act accumulates channels [0, a)
        acc_a = accp.tile([P, a], dt.float32, tag="acc_a")
        scr_a0 = scrp.tile([P, K], dt.bfloat16, tag="scr_a0")
        scr_a1 = scrp.tile([P, K], dt.bfloat16, tag="scr_a1")
        scr_as = [scr_a0, scr_a1]
        for i in range(a):
            nc.scalar.activation(
                out=scr_as[i % 2][:],
                in_=prod[:, i * K : (i + 1) * K],
                func=mybir.ActivationFunctionType.Copy,
                accum_out=acc_a[:, i : i + 1],
            )

    def tree(eng, cur, nch, seg, levels):
        """Binary add tree over the last axis of [P, nch, w0]; `levels`
        entries of TREE_WIDTHS starting where w0 = 2*levels[0]."""
        for li, w in enumerate(levels):
            t = scrp.tile([P, nch * w], dt.bfloat16, tag=f"tr{seg}{li}")
            tv = t[:].rearrange("q (c k) -> q c k", k=w)
            eng.tensor_tensor(
                out=tv, in0=cur[:, :, :w], in1=cur[:, :, w : 2 * w],
                op=AO.add,
            )
            cur = tv
        return cur

    if "pool" in parts and p:
        # Pool does the first POOL_LEVELS big tree levels for its channels
        # (gpsimd has ~0.8us fixed cost per instruction on real HW, so
        # minimize Pool instruction count); DVE finishes the tail.
        qp = tree(nc.gpsimd,
                  prod[:, AK : AK + PK].rearrange("q (c k) -> q c k", k=K),
                  p, "p", TREE_WIDTHS[:POOL_LEVELS])

    if "dve" in parts:
        # DVE add tree for channels [a+p, O): TT adds run in 2x bf16 mode
        # on real HW (TSP-accum measured 1x there - tree is 3x cheaper).
        acc_d = accp.tile([P, p + d], dt.float32, tag="acc_d")
        t9d = tree(nc.vector,
                   prod[:, AK + PK :].rearrange("q (c k) -> q c k", k=K),
                   d, "d", TREE_WIDTHS)
        nc.vector.tensor_reduce(out=acc_d[:, p : p + d], in_=t9d,
                                axis=mybir.AxisListType.X, op=AO.add)
        if "pool" in parts and p:
            t9p = tree(nc.vector, qp, p, "pc", TREE_WIDTHS[POOL_LEVELS:])
            nc.vector.tensor_reduce(out=acc_d[:, 0:p], in_=t9p,
                                    axis=mybir.AxisListType.X, op=AO.add)

    # ReLU: p+d channels on DVE (one TSP); a-channels joined onto DVE too
    # (DVE trails Act per block, so the join is usually free; putting it
    # on Pool would couple Pool to Act and serialize the pipeline).
    if "dve" in parts:
        if "pool" in parts and p:
            nc.vector.tensor_scalar_max(out=ob[:, a:O], in0=acc_d[:], scalar1=0.0)
        else:
            nc.vector.tensor_scalar_max(
                out=ob[:, a + p : O], in0=acc_d[:, p : p + d], scalar1=0.0
            )
    if "act" in parts:
        eng = nc.vector if "dve" in parts else nc.gpsimd
        eng.tensor_scalar_max(out=ob[:, 0:a], in0=acc_a[:], scalar1=0.0)


def _emit_block_split(nc, tc, mybir, dt, scrp, accp, ft, fe, ob):
    """3-engine reduction split fallback (DVE mults; Act/Pool/DVE reduce)."""
    a, p, d = A_ACT, P_POOL, D_TTR
    AK, PK = a * K, p * K

    def mult(prod_view, ft_view, nch):
        nc.vector.tensor_tensor(
            out=prod_view.rearrange("q (c k) -> q c k", k=K),
            in0=ft_view.rearrange("q (c k) -> q c k", k=K),
            in1=fe.broadcast_to((P, K, nch)).rearrange("q k c -> q c k"),
            op=mybir.AluOpType.mult,
        )

    def tree(eng, prod_view, nch, seg):
        cur = prod_view
        for li, w in enumerate(TREE_WIDTHS):
            t = scrp.tile([P, nch * w], dt.bfloat16, tag=f"tr{seg}{li}")
            tv = t[:].rearrange("q (c k) -> q c k", k=w)
            eng.tensor_tensor(
                out=tv, in0=cur[:, :, :w], in1=cur[:, :, w : 2 * w],
                op=mybir.AluOpType.add,
            )
            cur = tv
        return cur

    prod = scrp.tile([P, O * K], dt.bfloat16, tag="prod")
    mult(prod[:, :AK], ft[:, :AK], a)
    mult(prod[:, AK : AK + PK], ft[:, AK : AK + PK], p)
    mult(prod[:, AK + PK :], ft[:, AK + PK :], d)

    # Act accumulates channels [0, a)
    acc_a = accp.tile([P, a], dt.float32, tag="acc_a")
    scr_a = scrp.tile([P, K], dt.bfloat16, tag="scr_a")
    for i in range(a):
        nc.scalar.activation(
            out=scr_a[:],
            in_=prod[:, i * K : (i + 1) * K],
            func=mybir.ActivationFunctionType.Copy,
            accum_out=acc_a[:, i : i + 1],
        )

    # Pool tree for channels [a, a+p), finished on Pool
    q9 = tree(nc.gpsimd, prod[:, AK : AK + PK].rearrange("q (c k) -> q c k", k=K),
              p, "p")
    q4 = scrp.tile([P, p * 4], dt.bfloat16, tag="q4")
    q4v = q4[:].rearrange("q (c k) -> q c k", k=4)
    nc.gpsimd.tensor_tensor(out=q4v, in0=q9[:, :, 0:4], in1=q9[:, :, 4:8],
                            op=mybir.AluOpType.add)
    q2 = scrp.tile([P, p * 2], dt.bfloat16, tag="q2")
    q2v = q2[:].rearrange("q (c k) -> q c k", k=2)
    nc.gpsimd.tensor_tensor(out=q2v, in0=q4v[:, :, 0:2], in1=q4v[:, :, 2:4],
                            op=mybir.AluOpType.add)
    q1 = scrp.tile([P, p], dt.bfloat16, tag="q1")
    q1v = q1[:].rearrange("q (c k) -> q c k", k=1)
    nc.gpsimd.tensor_tensor(out=q1v, in0=q2v[:, :, 0:1], in1=q2v[:, :, 1:2],
                            op=mybir.AluOpType.add)
    qa = scrp.tile([P, p], dt.bfloat16, tag="qa")
    qav = qa[:].rearrange("q (c k) -> q c k", k=1)
    nc.gpsimd.tensor_tensor(out=qav, in0=q1v, in1=q9[:, :, 8:9],
                            op=mybir.AluOpType.add)

    # DVE tree for channels [a+p, O)
    acc_d = accp.tile([P, d], dt.float32, tag="acc_d")
    t9 = tree(nc.vector, prod[:, AK + PK :].rearrange("q (c k) -> q c k", k=K),
              d, "d")
    nc.vector.tensor_reduce(out=acc_d[:], in_=t9, axis=mybir.AxisListType.X,
                            op=mybir.AluOpType.add)

    # ReLU on the owning engine (never DVE<-Act/Pool)
    nc.vector.tensor_scalar_max(out=ob[:, a + p : O], in0=acc_d[:], scalar1=0.0)
    nc.gpsimd.tensor_scalar_max(out=ob[:, a : a + p], in0=qa[:], scalar1=0.0)
    nc.gpsimd.tensor_scalar_max(out=ob[:, 0:a], in0=acc_a[:], scalar1=0.0)


def _build_null_nc():
    """Same ExternalInput/Output signature as _build_nc, minimal work.

    Used by test.py to subtract input-upload + dispatch overhead from the
    wall-clock SPMD time (no NTFF profiling hook under this axon build).
    """
    from concourse import bacc, tile, mybir

    nc = bacc.Bacc("TRN2", debug=False)
    dt = mybir.dt

    nc.dram_tensor("filt", [LSH, O * K], dt.bfloat16, kind="ExternalInput")
    feat = nc.dram_tensor("feat", [LSH, K], dt.bfloat16, kind="ExternalInput")
    out = nc.dram_tensor("out", [LSH, O], dt.float32, kind="ExternalOutput")

    with tile.TileContext(nc) as tc:
        with tc.tile_pool(name="np_", bufs=1) as pool:
            t = pool.tile([P, O], dt.float32, tag="t")
            nc.vector.memset(t[:], 0.0)
            nc.sync.dma_start(out=out.ap()[0:P, :], in_=t[:])
            # touch feat so the input isn't pruned
            tf = pool.tile([P, 8], dt.bfloat16, tag="tf")
            nc.sync.dma_start(out=tf[:], in_=feat.ap()[0:P, 0:8])
    nc.compile()
    return nc


def _unfold_np(x):
    """numpy mirror of the reference unfold: [N,C,H,W] -> [N, L, C*9]."""
    xp = np.pad(x, ((0, 0), (0, 0), (1, 1), (1, 1)))
    patches = [
        xp[:, :, i : i + H, j : j + W] for i in range(KSZ) for j in range(KSZ)
    ]
    unf = np.stack(patches, axis=2)          # [N, C, 9, H, W]
    unf = unf.reshape(N, K, L)               # k = c*9 + (kh*3+kw)
    return unf.transpose(0, 2, 1)            # [N, L, K]


def kernel(features: np.ndarray, filters: np.ndarray) -> np.ndarray:
    from concourse.bass_utils import run_bass_kernel_spmd

    features = np.asarray(features, dtype=np.float32)
    filters = np.asarray(filters, dtype=np.float32)

    feat_unf = _unfold_np(features)          # [N, L, K] f32
    filt_bf = filters.astype(BF16)           # [N, L, K, O]

    in_maps = []
    for core in range(NCORES):
        n, q = divmod(core, NCORES // N)
        sl = slice(q * LSH, (q + 1) * LSH)
        fe = np.ascontiguousarray(feat_unf[n, sl]).astype(BF16)
        # o-major: per location, filter matrix transposed to [O, K]
        ftT = np.ascontiguousarray(
            filt_bf[n, sl].transpose(0, 2, 1)
        ).reshape(LSH, O * K)
        in_maps.append({"filt": ftT, "feat": fe})

    if "nc" not in _CACHE:
        _CACHE["nc"] = _build_nc()
    _CACHE["in_maps"] = in_maps
    res = run_bass_kernel_spmd(
        _CACHE["nc"], in_maps, list(range(NCORES)), trace=TRACE, **TRACE_KW
    )
    _CACHE["last_result"] = res

    out = np.empty((N, O, H, W), np.float32)
    out_flat = out.reshape(N, O, L)
    for core in range(NCORES):
        n, q = divmod(core, NCORES // N)
        o = np.asarray(res.results[core]["out"], dtype=np.float32)  # [LSH, O]
        out_flat[n, :, q * LSH : (q + 1) * LSH] = o.T
    return out
